# revision 1
# baseline (speedup 1.0000x reference)
"""LoRA Linear (T=8192, D_in=D_out=4096, r=16) on 8 TRN2 NeuronCores.

out = x @ W^T + b + (32/16) * ((x_bf16 @ A^T) @ B^T)

Strategy: data-parallel over the 8192-token axis (1024 tokens/core).
Host pre-transposes operands so the contraction dim d lands on SBUF
partitions with perfectly contiguous DMA:
  xT  [4096, 1024] fp32  (per-core shard, SBUF-resident, stationary operand)
  WT  [4096, 4096] fp32  (replicated, streamed once per core, moving operand)
Base matmul runs as float32r (fp32 truncated to ~FP22 in the PE) which is
full-rate when the moving free dim >= 256 -- vs 4x slower true fp32.
LoRA: lora1^T = A @ x^T computed first (fp32r, rank-16 output), rounded to
bf16 (matching the reference's bf16 intermediate), then the rank-16
expansion matmul (bf16) seeds each PSUM accumulation group before the 32
base-matmul accumulations; bias is added on the PSUM->SBUF copy (DVE).
LoRA scaling (32/16 = 2.0) is folded into B^T on the host (exact in bf16).
"""

import numpy as np

try:
    import concourse  # noqa: F401
except ImportError:  # pragma: no cover
    import sys

    sys.path.insert(0, "/opt/trn_rl_repo")

from concourse import bacc, mybir, tile
from concourse.bass_utils import run_bass_kernel_spmd

N_CORES = 8
T, D_IN, D_OUT, R = 8192, 4096, 4096, 16
TPC = T // N_CORES  # 1024 tokens per core
N_DC = D_IN // 128  # 32 contraction chunks of 128
OC = 512  # output-column chunk (one PSUM bank of fp32)
N_OC = D_OUT // OC  # 8
N_TC = TPC // 128  # 8 token tiles of 128

f32 = mybir.dt.float32
f32r = mybir.dt.float32r
bf16 = mybir.dt.bfloat16

_NC_CACHE = {}


def build_nc(reps=1, loop_reps=0, w_once=False, x_bf16=False):
    xdt = bf16 if x_bf16 else f32r
    nc = bacc.Bacc(
        "TRN2", target_bir_lowering=False, debug=False, num_devices=N_CORES
    )
    xT = nc.dram_tensor("xT", [D_IN, TPC], xdt, kind="ExternalInput").ap()
    WT = nc.dram_tensor("WT", [D_IN, D_OUT], f32r, kind="ExternalInput").ap()
    AT = nc.dram_tensor("AT", [D_IN, R], xdt, kind="ExternalInput").ap()
    BT = nc.dram_tensor("BT", [R, D_OUT], bf16, kind="ExternalInput").ap()
    bias = nc.dram_tensor("bias", [128, D_OUT], f32, kind="ExternalInput").ap()
    out = nc.dram_tensor("out", [TPC, D_OUT], f32, kind="ExternalOutput").ap()

    with tile.TileContext(nc) as tc:
        with (
            tc.tile_pool(name="persist", bufs=1) as persist,
            tc.tile_pool(name="xpool", bufs=N_DC) as xpool,
            tc.tile_pool(name="wpool", bufs=4) as wpool,
            tc.tile_pool(name="opool", bufs=6) as opool,
            tc.tile_pool(name="pspool", bufs=8, space="PSUM") as pspool,
        ):
          def _emit_body():
            at_sb = persist.tile([128, N_DC * R], xdt, tag="at")
            bt_sb = persist.tile([R, D_OUT], bf16, tag="bt")
            bias_sb = persist.tile([128, D_OUT], f32, tag="bias")
            lora1_sb = persist.tile([R, TPC], bf16, tag="lora1")

            nc.sync.dma_start(out=bias_sb[:], in_=bias[:])
            nc.sync.dma_start(out=bt_sb[:], in_=BT[:])
            for dc in range(N_DC):
                nc.sync.dma_start(
                    out=at_sb[:, dc * R : (dc + 1) * R],
                    in_=AT[dc * 128 : (dc + 1) * 128, :],
                )

            xt_tiles = []
            for dc in range(N_DC):
                xt = xpool.tile([128, TPC], xdt, tag="xt")
                nc.sync.dma_start(
                    out=xt[:], in_=xT[dc * 128 : (dc + 1) * 128, :]
                )
                xt_tiles.append(xt)

            # Phase 1: lora1T[r, t] = sum_d A[r, d] * x[t, d]  (fp32r),
            # rounded to bf16 like the reference's bf16 einsum output.
            for th in range(TPC // OC):
                ps_l = pspool.tile([R, OC], f32, tag="ps")
                for dc in range(N_DC):
                    nc.tensor.matmul(
                        ps_l[:],
                        at_sb[:, dc * R : (dc + 1) * R],
                        xt_tiles[dc][:, th * OC : (th + 1) * OC],
                        start=(dc == 0),
                        stop=(dc == N_DC - 1),
                    )
                nc.vector.tensor_copy(
                    lora1_sb[:, th * OC : (th + 1) * OC], ps_l[:]
                )

            # Phase 2: out[t, o] = lora2 + sum_d x[t, d] W[o, d] + bias
            for oc in range(N_OC):
                osl = slice(oc * OC, (oc + 1) * OC)
                ps_tiles = [
                    pspool.tile([128, OC], f32, tag="ps", name=f"ps_{oc}_{t}")
                    for t in range(N_TC)
                ]
                # Seed each accumulation group with the rank-16 LoRA matmul.
                for t in range(N_TC):
                    nc.tensor.matmul(
                        ps_tiles[t][:],
                        lora1_sb[:, t * 128 : (t + 1) * 128],
                        bt_sb[:, osl],
                        start=True,
                        stop=False,
                    )
                if w_once:
                    wt0 = wpool.tile([128, OC], f32r, tag="wt", name=f"wto{oc}")
                    nc.sync.dma_start(out=wt0[:], in_=WT[0:128, osl])
                for dc in range(N_DC):
                    if w_once:
                        wt = wt0
                    else:
                        wt = wpool.tile([128, OC], f32r, tag="wt")
                        nc.sync.dma_start(
                            out=wt[:], in_=WT[dc * 128 : (dc + 1) * 128, osl]
                        )
                    for t in range(N_TC):
                        nc.tensor.matmul(
                            ps_tiles[t][:],
                            xt_tiles[dc][:, t * 128 : (t + 1) * 128],
                            wt[:],
                            start=False,
                            stop=(dc == N_DC - 1),
                        )
                for t in range(N_TC):
                    o_sb = opool.tile([128, OC], f32, tag="osb")
                    nc.vector.tensor_tensor(
                        o_sb[:],
                        ps_tiles[t][:],
                        bias_sb[:, osl],
                        mybir.AluOpType.add,
                    )
                    nc.sync.dma_start(
                        out=out[t * 128 : (t + 1) * 128, osl], in_=o_sb[:]
                    )

          if loop_reps:
              with tc.For_i(0, loop_reps, 1):
                  _emit_body()
          else:
              for _rep in range(reps):
                  _emit_body()

    nc.compile()
    return nc


def _prepare_in_maps(x, W, b, lora_a, lora_b, x_bf16=False, w_scale=1.0):
    import ml_dtypes

    xdt = ml_dtypes.bfloat16 if x_bf16 else np.float32
    WT = np.ascontiguousarray(W.T)  # [D_IN, D_OUT] fp32
    if w_scale != 1.0:
        WT = WT * np.float32(w_scale)
    AT = np.ascontiguousarray(lora_a.T).astype(xdt)  # [D_IN, R]
    # Fold the LoRA scaling (alpha/r = 2.0) into B^T; exact in bf16.
    BT = (np.ascontiguousarray(lora_b.T).astype(np.float32) * 2.0).astype(
        ml_dtypes.bfloat16
    )  # [R, D_OUT]
    bias = np.ascontiguousarray(
        np.broadcast_to(b.astype(np.float32), (128, D_OUT))
    )
    in_maps = []
    for c in range(N_CORES):
        xTc = np.ascontiguousarray(x[c * TPC : (c + 1) * TPC].T).astype(xdt)
        in_maps.append(
            {"xT": xTc, "WT": WT, "AT": AT, "BT": BT, "bias": bias}
        )
    return in_maps


def run(inputs, trace=False, **trace_kwargs):
    """Run on hardware; returns (full_output, BassKernelResults)."""
    if "nc" not in _NC_CACHE:
        _NC_CACHE["nc"] = build_nc()
    nc = _NC_CACHE["nc"]
    in_maps = _prepare_in_maps(
        np.asarray(inputs["x"], dtype=np.float32),
        np.asarray(inputs["W"], dtype=np.float32),
        np.asarray(inputs["b"], dtype=np.float32),
        np.asarray(inputs["lora_a"]),
        np.asarray(inputs["lora_b"]),
    )
    res = run_bass_kernel_spmd(
        nc, in_maps, list(range(N_CORES)), trace=trace, **trace_kwargs
    )
    out = np.concatenate(
        [res.results[c]["out"] for c in range(N_CORES)], axis=0
    )
    return out.astype(np.float32), res


def kernel(**inputs):
    out, _ = run(inputs, trace=False)
    return out


if __name__ == "__main__":
    rng = np.random.default_rng(0)
    import ml_dtypes

    x = rng.standard_normal((T, D_IN), dtype=np.float32)
    W = rng.standard_normal((D_OUT, D_IN), dtype=np.float32) * 0.02
    b = rng.standard_normal((D_OUT,), dtype=np.float32) * 0.02
    la = (rng.standard_normal((R, D_IN), dtype=np.float32) * 0.02).astype(
        ml_dtypes.bfloat16
    )
    lb = (rng.standard_normal((D_OUT, R), dtype=np.float32) * 0.02).astype(
        ml_dtypes.bfloat16
    )
    got = kernel(x=x, W=W, b=b, lora_a=la, lora_b=lb)
    ref = (
        x @ W.T
        + b
        + 2.0
        * (
            (x.astype(ml_dtypes.bfloat16).astype(np.float32) @ la.astype(np.float32).T)
            @ lb.astype(np.float32).T
        )
    )
    err = np.abs(got - ref).max() / np.abs(ref).max()
    print("scale-relative max err:", err)



# revision 3
# speedup vs baseline: 1.0117x; 1.0117x over previous
"""LoRA Linear (T=8192, D_in=D_out=4096, r=16) on 8 TRN2 NeuronCores.

out = x @ W^T + b + (32/16) * ((x_bf16 @ A^T) @ B^T)

Strategy: data-parallel over tokens (1024 tokens/core), with the LoRA
path folded into the weight on the HOST: W' = W + 2.0 * (B @ A).  The
device then runs a single dense bf16 GEMM out = x @ W'^T + bias.

Device-side layout (per core):
  xT  [4096 d, 1024 t] bf16  SBUF-resident, stationary operand
  WT  [4096 d, 4096 o] bf16  streamed once in four o-quarters
  out [1024 t, 4096 o] bf16  (bias added on PSUM->SBUF evac, DVE)

Each matmul: stationary xT-tile [128 d, 128 t], moving WT-slab
[128 d, 1024 o] (bf16 moving max), accumulating 32 d-chunks into a
2-bank PSUM tile [128 t, 1024 o] fp32.  bf16 weights get FWL fast
weight loads, so LDWEIGHTS hides under the 1024-cycle matmuls.
"""

import numpy as np

try:
    import concourse  # noqa: F401
except ImportError:  # pragma: no cover
    import sys

    sys.path.insert(0, "/opt/trn_rl_repo")

from concourse import bacc, mybir, tile
from concourse.bass_utils import run_bass_kernel_spmd

N_CORES = 8
T, D_IN, D_OUT, R = 8192, 4096, 4096, 16
TPC = T // N_CORES  # 1024 tokens per core
N_DC = D_IN // 128  # 32 contraction chunks of 128
OQ = 1024  # output-column quarter (moving free dim, bf16 max)
N_OQ = D_OUT // OQ  # 4
N_TC = TPC // 128  # 8 token tiles of 128

f32 = mybir.dt.float32
bf16 = mybir.dt.bfloat16

_NC_CACHE = {}


def build_nc(split_mm=True, w_bufs=48):
    nc = bacc.Bacc(
        "TRN2", target_bir_lowering=False, debug=False, num_devices=N_CORES
    )
    xT = nc.dram_tensor("xT", [D_IN, TPC], bf16, kind="ExternalInput").ap()
    WT = nc.dram_tensor("WT", [D_IN, D_OUT], bf16, kind="ExternalInput").ap()
    bias = nc.dram_tensor("bias", [128, D_OUT], f32, kind="ExternalInput").ap()
    out = nc.dram_tensor("out", [TPC, D_OUT], bf16, kind="ExternalOutput").ap()

    with tile.TileContext(nc) as tc:
        with (
            tc.tile_pool(name="persist", bufs=1) as persist,
            tc.tile_pool(name="xpool", bufs=N_DC) as xpool,
            tc.tile_pool(name="wpool", bufs=w_bufs) as wpool,
            tc.tile_pool(name="opool", bufs=4) as opool,
            tc.tile_pool(name="pspool", bufs=4, space="PSUM") as pspool,
        ):
            bias_sb = persist.tile([128, D_OUT], f32, tag="bias")
            nc.sync.dma_start(out=bias_sb[:], in_=bias[:])

            xt_tiles = []
            for dc in range(N_DC):
                xt = xpool.tile([128, TPC], bf16, tag="xt")
                nc.sync.dma_start(
                    out=xt[:], in_=xT[dc * 128 : (dc + 1) * 128, :]
                )
                xt_tiles.append(xt)

            for oq in range(N_OQ):
                osl = slice(oq * OQ, (oq + 1) * OQ)
                w_slabs = []
                for dc in range(N_DC):
                    w = wpool.tile([128, OQ], bf16, tag="wt")
                    nc.sync.dma_start(
                        out=w[:], in_=WT[dc * 128 : (dc + 1) * 128, osl]
                    )
                    w_slabs.append(w)
                for tc_i in range(N_TC):
                    ps = pspool.tile([128, OQ], f32, tag="ps")
                    tsl = slice(tc_i * 128, (tc_i + 1) * 128)
                    for dc in range(N_DC):
                        if split_mm:
                            for h in range(2):
                                hs = slice(h * 512, (h + 1) * 512)
                                nc.tensor.matmul(
                                    ps[:, hs],
                                    xt_tiles[dc][:, tsl],
                                    w_slabs[dc][:, hs],
                                    start=(dc == 0),
                                    stop=(dc == N_DC - 1),
                                )
                        else:
                            nc.tensor.matmul(
                                ps[:],
                                xt_tiles[dc][:, tsl],
                                w_slabs[dc][:],
                                start=(dc == 0),
                                stop=(dc == N_DC - 1),
                            )
                    o_sb = opool.tile([128, OQ], bf16, tag="osb")
                    nc.vector.tensor_tensor(
                        o_sb[:], ps[:], bias_sb[:, osl], mybir.AluOpType.add
                    )
                    nc.sync.dma_start(out=out[tsl, osl], in_=o_sb[:])

    nc.compile()
    return nc


def _prepare_in_maps(x, W, b, lora_a, lora_b):
    import ml_dtypes

    # Fold the LoRA path (scaling alpha/r = 2.0) into the weight.
    Wp = W + 2.0 * (
        lora_b.astype(np.float32) @ lora_a.astype(np.float32)
    )  # [D_OUT, D_IN] fp32
    WT = np.ascontiguousarray(Wp.T).astype(ml_dtypes.bfloat16)  # [D_IN, D_OUT]
    bias = np.ascontiguousarray(
        np.broadcast_to(b.astype(np.float32), (128, D_OUT))
    )
    in_maps = []
    for c in range(N_CORES):
        xTc = np.ascontiguousarray(x[c * TPC : (c + 1) * TPC].T).astype(
            ml_dtypes.bfloat16
        )
        in_maps.append({"xT": xTc, "WT": WT, "bias": bias})
    return in_maps


def run(inputs, trace=False, **trace_kwargs):
    """Run on hardware; returns (full_output, BassKernelResults)."""
    if "nc" not in _NC_CACHE:
        _NC_CACHE["nc"] = build_nc()
    nc = _NC_CACHE["nc"]
    in_maps = _prepare_in_maps(
        np.asarray(inputs["x"], dtype=np.float32),
        np.asarray(inputs["W"], dtype=np.float32),
        np.asarray(inputs["b"], dtype=np.float32),
        np.asarray(inputs["lora_a"]).astype(np.float32),
        np.asarray(inputs["lora_b"]).astype(np.float32),
    )
    res = run_bass_kernel_spmd(
        nc, in_maps, list(range(N_CORES)), trace=trace, **trace_kwargs
    )
    out = np.concatenate(
        [res.results[c]["out"] for c in range(N_CORES)], axis=0
    )
    return out.astype(np.float32), res


def kernel(**inputs):
    out, _ = run(inputs, trace=False)
    return out


if __name__ == "__main__":
    rng = np.random.default_rng(0)
    import ml_dtypes

    x = rng.standard_normal((T, D_IN), dtype=np.float32)
    W = rng.standard_normal((D_OUT, D_IN), dtype=np.float32) * 0.02
    b = rng.standard_normal((D_OUT,), dtype=np.float32) * 0.02
    la = (rng.standard_normal((R, D_IN), dtype=np.float32) * 0.02).astype(
        ml_dtypes.bfloat16
    )
    lb = (rng.standard_normal((D_OUT, R), dtype=np.float32) * 0.02).astype(
        ml_dtypes.bfloat16
    )
    got = kernel(x=x, W=W, b=b, lora_a=la, lora_b=lb)
    ref = (
        x @ W.T
        + b
        + 2.0
        * (
            (x @ la.astype(np.float32).T)
            @ lb.astype(np.float32).T
        )
    )
    err = np.abs(got - ref).max() / np.abs(ref).max()
    print("scale-relative max err:", err)


# revision 4
# speedup vs baseline: 1.2353x; 1.2211x over previous
"""LoRA Linear (T=8192, D_in=D_out=4096, r=16) on 8 TRN2 NeuronCores.

out = x @ W^T + b + (32/16) * ((x_bf16 @ A^T) @ B^T)

Strategy: data-parallel over tokens (1024 tokens/core), with the LoRA
path folded into the weight on the HOST: W' = W + 2.0 * (B @ A).  The
device then runs a single dense bf16 GEMM: out = x @ W'^T + bias.

Device-side layout (per core):
  xT  [4096 d, 1024 t] bf16  SBUF-resident, stationary operand
  WT  [4096 d, 4096 o] bf16  streamed once, in four 1024-column o-phases
  out [1024 t, 4096 o] bf16  (bias added on the PSUM->SBUF evac, DVE)

Each matmul: stationary xT-tile [128 d, 128 t], moving W slab
[128 d, 512 o].  W slabs are standalone [128, 512] SBUF tiles: the
moving operand must be a fully-contiguous AP -- streaming a 512-column
slice out of a wider tile (strided rows) costs ~20% matmul throughput.
Accumulation: 32 d-chunks into one PSUM bank [128 t, 512 o] fp32; the
two 512-halves of a phase use two banks.  2048 matmuls total, measured
~219 ns each (floor 216), PE-bound.

DMA choreography: x tiles and phase-0 W slabs interleaved at the start
(first matmul at ~14 us), bias DMA delayed a few slots, next phase's 64
W slabs emitted at the phase boundary.  112 W-slab bufs give a
1.75-phase rolling window.  SBUF ~25 MB of 26.6 usable.
"""

import numpy as np

try:
    import concourse  # noqa: F401
except ImportError:  # pragma: no cover
    import sys

    sys.path.insert(0, "/opt/trn_rl_repo")

from concourse import bacc, mybir, tile
from concourse.bass_utils import run_bass_kernel_spmd

N_CORES = 8
T, D_IN, D_OUT, R = 8192, 4096, 4096, 16
TPC = T // N_CORES  # 1024 tokens per core
N_DC = D_IN // 128  # 32 contraction chunks of 128
N_TC = TPC // 128  # 8 token tiles of 128
OC = 512  # one PSUM bank of fp32; moving free dim per matmul
N_PH = 4  # o-phases
PH_OC = D_OUT // N_PH  # 1024 columns per phase
W_BUFS = 112

f32 = mybir.dt.float32
bf16 = mybir.dt.bfloat16

_NC_CACHE = {}


def build_nc():
    nc = bacc.Bacc(
        "TRN2", target_bir_lowering=False, debug=False, num_devices=N_CORES
    )
    xT = nc.dram_tensor("xT", [D_IN, TPC], bf16, kind="ExternalInput").ap()
    WT = nc.dram_tensor("WT", [D_IN, D_OUT], bf16, kind="ExternalInput").ap()
    bias = nc.dram_tensor("bias", [128, D_OUT], f32, kind="ExternalInput").ap()
    out = nc.dram_tensor("out", [TPC, D_OUT], bf16, kind="ExternalOutput").ap()

    with tile.TileContext(nc) as tc:
        with (
            tc.tile_pool(name="persist", bufs=1) as persist,
            tc.tile_pool(name="xpool", bufs=N_DC) as xpool,
            tc.tile_pool(name="wpool", bufs=W_BUFS) as wpool,
            tc.tile_pool(name="opool", bufs=4) as opool,
            tc.tile_pool(name="pspool", bufs=8, space="PSUM") as pspool,
        ):
            w_slabs = {}

            def emit_w_slab(ph, k):
                dc, j = k // 2, k % 2
                w = wpool.tile([128, OC], bf16, tag="wt", name=f"w_{ph}_{k}")
                o0 = ph * PH_OC + j * OC
                nc.sync.dma_start(
                    out=w[:],
                    in_=WT[dc * 128 : (dc + 1) * 128, o0 : o0 + OC],
                )
                w_slabs[(ph, dc, j)] = w

            bias_sb = persist.tile([128, D_OUT], f32, tag="bias")
            xt_tiles = []
            for dc in range(N_DC):
                xt = xpool.tile([128, TPC], bf16, tag="xt")
                nc.sync.dma_start(
                    out=xt[:], in_=xT[dc * 128 : (dc + 1) * 128, :]
                )
                xt_tiles.append(xt)
                emit_w_slab(0, 2 * dc)
                emit_w_slab(0, 2 * dc + 1)
                if dc == 7:
                    nc.sync.dma_start(out=bias_sb[:], in_=bias[:])

            for ph in range(N_PH):
                for tc_i in range(N_TC):
                    tsl = slice(tc_i * 128, (tc_i + 1) * 128)
                    ps_tiles = [
                        pspool.tile(
                            [128, OC], f32, tag="ps", name=f"ps_{ph}_{tc_i}_{k}"
                        )
                        for k in range(2)
                    ]
                    for dc in range(N_DC):
                        for j in range(2):
                            nc.tensor.matmul(
                                ps_tiles[j][:],
                                xt_tiles[dc][:, tsl],
                                w_slabs[(ph, dc, j)][:],
                                start=(dc == 0),
                                stop=(dc == N_DC - 1),
                            )
                    for j in range(2):
                        o_sb = opool.tile([128, OC], bf16, tag="osb")
                        o0 = ph * PH_OC + j * OC
                        nc.vector.tensor_tensor(
                            o_sb[:],
                            ps_tiles[j][:],
                            bias_sb[:, o0 : o0 + OC],
                            mybir.AluOpType.add,
                        )
                        nc.sync.dma_start(
                            out=out[tsl, o0 : o0 + OC], in_=o_sb[:]
                        )
                if ph + 1 < N_PH:
                    for k in range(2 * N_DC):
                        emit_w_slab(ph + 1, k)

    nc.compile()
    return nc


def _prepare_in_maps(x, W, b, lora_a, lora_b):
    import ml_dtypes

    # Fold the LoRA path (scaling alpha/r = 2.0) into the weight.
    Wp = W + 2.0 * (
        lora_b.astype(np.float32) @ lora_a.astype(np.float32)
    )  # [D_OUT, D_IN] fp32
    WT = np.ascontiguousarray(Wp.T).astype(ml_dtypes.bfloat16)  # [D_IN, D_OUT]
    bias = np.ascontiguousarray(
        np.broadcast_to(b.astype(np.float32), (128, D_OUT))
    )
    in_maps = []
    for c in range(N_CORES):
        xTc = np.ascontiguousarray(x[c * TPC : (c + 1) * TPC].T).astype(
            ml_dtypes.bfloat16
        )
        in_maps.append({"xT": xTc, "WT": WT, "bias": bias})
    return in_maps


def run(inputs, trace=False, **trace_kwargs):
    """Run on hardware; returns (full_output, BassKernelResults)."""
    if "nc" not in _NC_CACHE:
        _NC_CACHE["nc"] = build_nc()
    nc = _NC_CACHE["nc"]
    in_maps = _prepare_in_maps(
        np.asarray(inputs["x"], dtype=np.float32),
        np.asarray(inputs["W"], dtype=np.float32),
        np.asarray(inputs["b"], dtype=np.float32),
        np.asarray(inputs["lora_a"]).astype(np.float32),
        np.asarray(inputs["lora_b"]).astype(np.float32),
    )
    res = run_bass_kernel_spmd(
        nc, in_maps, list(range(N_CORES)), trace=trace, **trace_kwargs
    )
    out = np.concatenate(
        [res.results[c]["out"] for c in range(N_CORES)], axis=0
    )
    return out.astype(np.float32), res


def kernel(**inputs):
    out, _ = run(inputs, trace=False)
    return out


if __name__ == "__main__":
    rng = np.random.default_rng(0)
    import ml_dtypes

    x = rng.standard_normal((T, D_IN), dtype=np.float32)
    W = rng.standard_normal((D_OUT, D_IN), dtype=np.float32) * 0.02
    b = rng.standard_normal((D_OUT,), dtype=np.float32) * 0.02
    la = (rng.standard_normal((R, D_IN), dtype=np.float32) * 0.02).astype(
        ml_dtypes.bfloat16
    )
    lb = (rng.standard_normal((D_OUT, R), dtype=np.float32) * 0.02).astype(
        ml_dtypes.bfloat16
    )
    got = kernel(x=x, W=W, b=b, lora_a=la, lora_b=lb)
    ref = (
        x @ W.T
        + b
        + 2.0 * ((x @ la.astype(np.float32).T) @ lb.astype(np.float32).T)
    )
    err = np.abs(got - ref).max() / np.abs(ref).max()
    print("scale-relative max err:", err)
